# revision 27
# baseline (speedup 1.0000x reference)
"""Trainium2 Bass kernel for nn_GroupDenseFull.

Math: z[b, t*8+v] = sum_{s,w} x[b, s*8+w] * kernel_seq[s,w,v] * kernel_full[s,t]

Instead of materializing the dense 1024x1024 combined weight (275 GFLOP), use
the factored form (36.5 GFLOP):
  step1 (grouped): y[b,s,v] = sum_w x[b,s,w] * ks[s,w,v]
  step2 (mixing):  z[b,t,v] = sum_s y[b,s,v] * kf[s,t]

Sharding: data-parallel over batch across 8 cores (16384 rows each).

Per-core pipeline (bf16 I/O to halve HBM traffic; rel err ~4e-3 << 2e-2):
  1. xbar-transpose DMA load: x chunk [2048b x 1024c] -> xT [128c x 8k x 2048b]
     (one contiguous 4MB source per superchunk; HW transposes during DMA).
  2. step1 per (k, j): matmul(lhsT=xT block [c x b], rhs=A_k) -> y batch-major.
     A_k is the 128x128 block-diagonal grouped weight, with output columns
     ordered (v, s_l) so the eviction assembles yB with v-major columns
     (global col = v*128 + s).
  3. tY per v: PE transpose of yB v-block [128b x 128s] -> yT_v [s x b].
  4. step2 per v: matmul(lhsT=yT_v, rhs=kernel_full [s x t]) -> z[b, t]
     batch-major directly; evicted into zB columns t*8+v (stride-8 AP).
  5. contiguous bf16 store.
"""

import os
from contextlib import ExitStack

import numpy as np
import ml_dtypes

import concourse.bass as bass
import concourse.tile as tile
from concourse import bacc, mybir
from concourse.bass_utils import run_bass_kernel_spmd

B, C, W, S = 131072, 1024, 8, 128
NCORES = 8
BSH = B // NCORES          # 16384 rows per core
SC = 1024                  # superchunk rows (one 2MB transposed load)
NSC = BSH // SC            # 16 superchunks
NJ = SC // 128             # 8 batch subtiles per superchunk
NK = C // 128              # 8 channel tiles

F32 = mybir.dt.float32
BF16 = mybir.dt.bfloat16

TRACE = bool(int(os.environ.get("KERNEL_TRACE", "0")))
LAST_EXEC_NS = None
LAST_TRACE_DIR = None

_cache = {}


def _setup_trace_shim():
    """The agent image lacks antenv.axon_hooks; register the NTFF profile
    hook ourselves so run_bass_kernel_spmd(trace=True) works."""
    import sys
    import types

    import antenv
    from trn_agent_boot.trn_boot import _ntff_profile_via_ctypes

    if "antenv.axon_hooks" in sys.modules:
        return
    mod = types.ModuleType("antenv.axon_hooks")
    mod._hook = _ntff_profile_via_ctypes("/opt/axon/libaxon_pjrt.so")
    mod.get_axon_ntff_profile_hook = lambda: mod._hook
    mod.set_axon_ntff_profile_hook = lambda h: setattr(mod, "_hook", h)
    sys.modules["antenv.axon_hooks"] = mod
    antenv.axon_hooks = mod
    # no bucket in this container; keep artifacts local
    import concourse.bass_utils as bu

    bu.upload_artifacts = lambda tmpdir: tmpdir


def _build():
    nc = bacc.Bacc(
        "TRN2", target_bir_lowering=False, debug=False, num_devices=NCORES
    )
    # sc-major + k-major layout: each superchunk is one contiguous 4MB
    # block whose 128-col rows make the xbar transpose DMA's source tile
    # reads contiguous (fast M2S concat path)
    x_ap = nc.dram_tensor(
        "x", [NSC, NK * SC, 128], BF16, kind="ExternalInput"
    ).ap()
    a_ap = nc.dram_tensor("a", [NK, 128, 128], BF16, kind="ExternalInput").ap()
    kf_ap = nc.dram_tensor("kf", [128, 128], BF16, kind="ExternalInput").ap()
    id_ap = nc.dram_tensor("ident", [128, 128], BF16, kind="ExternalInput").ap()
    z_ap = nc.dram_tensor("z", [BSH, C], BF16, kind="ExternalOutput").ap()

    with tile.TileContext(nc) as tc, ExitStack() as ctx:
        consts = ctx.enter_context(tc.tile_pool(name="consts", bufs=1))
        ident = consts.tile([128, 128], BF16)
        nc.sync.dma_start(ident, id_ap)
        a_sb = consts.tile([128, NK, 128], BF16)
        nc.sync.dma_start(a_sb, a_ap.rearrange("k p c -> p k c"))
        kf_sb = consts.tile([128, 128], BF16)
        nc.sync.dma_start(kf_sb, kf_ap)

        xtpool = ctx.enter_context(tc.tile_pool(name="xt", bufs=3))
        ybpool = ctx.enter_context(tc.tile_pool(name="yb", bufs=3))
        ytpool = ctx.enter_context(tc.tile_pool(name="yt", bufs=3))
        zpool = ctx.enter_context(tc.tile_pool(name="zb", bufs=2))
        ps1 = ctx.enter_context(tc.tile_pool(name="ps1", bufs=2, space="PSUM"))
        pst = ctx.enter_context(tc.tile_pool(name="pst", bufs=2, space="PSUM"))
        ps2 = ctx.enter_context(tc.tile_pool(name="ps2", bufs=1, space="PSUM"))

        def load_xt(sc):
            xt = xtpool.tile([128, NK, SC], BF16, tag="xt", name="xt")
            nc.sync.dma_start_transpose(xt, x_ap[sc])
            return xt

        def step1(xt, j):
            """grouped matmul for subtile j -> yb [128b x (v, k, s_l)]
            (v-major columns: global col v*128 + 16k + s_l = v*128 + s)"""
            yb = ybpool.tile([128, 8, 8, 16], BF16, tag="yb")  # (v, k, s_l)
            p1 = ps1.tile([128, 8, 8, 16], F32, tag="p1")      # (k, v, s_l)
            for k in range(NK):
                nc.tensor.matmul(
                    p1[:, k, :, :],
                    xt[:, k, j * 128:(j + 1) * 128],
                    a_sb[:, k, :],
                )
            # f32 PSUM reads run at 1 elem/lane/cycle on both engines;
            # ACT takes ~2/3 of the f32 casts, DVE has the bf16 copies
            eng_copy(
                nc.vector if j % 3 == 0 else nc.scalar,
                out=yb,
                in_=p1.rearrange("p k v s -> p v k s"),
            )
            return yb

        def step2(yb, j, zb):
            """transpose each v-block to [s x b], then mix with kf"""
            ybf = yb.rearrange("p v k s -> p v (k s)")
            pt = pst.tile([128, 8, 128], BF16, tag="pt")
            for v in range(8):
                nc.tensor.transpose(pt[:, v, :], ybf[:, v, :], ident)
            yt = ytpool.tile([128, 8, 128], BF16, tag="yt")
            eng_copy(nc.vector, out=yt, in_=pt)  # bf16 copy: 2x only on DVE
            p2 = ps2.tile([128, 8, 128], F32, tag="p2")
            for v in range(8):
                nc.tensor.matmul(p2[:, v, :], yt[:, v, :], kf_sb)
            # zb stays (v, t)-major; host does the final (t*8+v) interleave
            eng_copy(
                nc.vector if j % 3 == 1 else nc.scalar,
                out=zb[:, j, :, :],
                in_=p2,
            )

        def eng_copy(eng, out, in_):
            if eng is nc.scalar:
                eng.copy(out=out, in_=in_)
            else:
                eng.tensor_copy(out=out, in_=in_)

        # flattened j-pipeline across superchunk boundaries: the PE always
        # has step1(j+1) queued before step2(j), even across sc edges
        PREFETCH = 2
        xts = [load_xt(sc) for sc in range(PREFETCH)]
        zbs = {}

        def get_zb(sc):
            if sc not in zbs:
                zbs[sc] = zpool.tile(
                    [128, NJ, 8, 128], BF16, tag="zb", name="zb"
                )
            return zbs[sc]

        total_j = NSC * NJ
        yb_next = step1(xts[0], 0)
        for ji in range(total_j):
            sc, j = divmod(ji, NJ)
            yb = yb_next
            if j == 0 and sc + PREFETCH < NSC:
                xts.append(load_xt(sc + PREFETCH))
            if ji + 1 < total_j:
                sc1, j1 = divmod(ji + 1, NJ)
                yb_next = step1(xts[sc1], j1)
            step2(yb, j, get_zb(sc))
            if j == NJ - 1:
                nc.scalar.dma_start(
                    z_ap[sc * SC:(sc + 1) * SC, :].rearrange(
                        "(j p) c -> p j c", p=128
                    ),
                    zbs.pop(sc).rearrange("p j v t -> p j (v t)"),
                )

    nc.compile()
    return nc


def _make_A(ks):
    """A_k[(s_l*8+w), (v*16+s_l)] = ks[16k+s_l, w, v] (block-diag grouped
    weight with (v, s_l)-ordered output columns)."""
    A = np.zeros((NK, 128, 128), np.float32)
    for k in range(NK):
        for sl in range(16):
            A[k, sl * 8:(sl + 1) * 8, sl::16] = ks[16 * k + sl]
    return A


def kernel(x, kernel_seq, kernel_full):
    global LAST_EXEC_NS
    x = np.asarray(x, dtype=np.float32)
    ks = np.asarray(kernel_seq, dtype=np.float32)
    kf = np.asarray(kernel_full, dtype=np.float32)

    xb = np.ascontiguousarray(x).astype(ml_dtypes.bfloat16)
    ab = _make_A(ks).astype(ml_dtypes.bfloat16)
    kfb = np.ascontiguousarray(kf).astype(ml_dtypes.bfloat16)
    ident = np.eye(128, dtype=ml_dtypes.bfloat16)

    if "nc" not in _cache:
        _cache["nc"] = _build()
    nc = _cache["nc"]

    # per-core [NSC, NK*SC, 128]: superchunk-major, k-major within
    xs = np.ascontiguousarray(
        xb.reshape(NCORES, NSC, SC, NK, 128).transpose(0, 1, 3, 2, 4)
    ).reshape(NCORES, NSC, NK * SC, 128)
    in_maps = [
        {"x": xs[i], "a": ab, "kf": kfb, "ident": ident} for i in range(NCORES)
    ]
    kw = {}
    if TRACE:
        _setup_trace_shim()
        global LAST_TRACE_DIR
        import tempfile

        LAST_TRACE_DIR = tempfile.mkdtemp(prefix="ktrace_")
        kw = {"tmpdir": LAST_TRACE_DIR}
    res = run_bass_kernel_spmd(nc, in_maps, list(range(NCORES)), trace=TRACE, **kw)
    if res.exec_time_ns is not None:
        LAST_EXEC_NS = res.exec_time_ns
    z = np.concatenate([r["z"] for r in res.results], axis=0)
    # device stores (v, t)-major columns; natural layout is c = t*8 + v
    z = z.reshape(B, 8, 128).transpose(0, 2, 1).reshape(B, C)
    return np.ascontiguousarray(z.astype(np.float32))


# revision 31
# speedup vs baseline: 1.5216x; 1.5216x over previous
"""Trainium2 Bass kernel for nn_GroupDenseFull.

Math: z[b, t*8+v] = sum_{s,w} x[b, s*8+w] * kernel_seq[s,w,v] * kernel_full[s,t]

Instead of materializing the dense 1024x1024 combined weight (275 GFLOP), use
the factored form (36.5 GFLOP):
  step1 (grouped): y[b,s,v] = sum_w x[b,s,w] * ks[s,w,v]
  step2 (mixing):  z[b,t,v] = sum_s y[b,s,v] * kf[s,t]

Sharding: data-parallel over batch across 8 cores (16384 rows each).

Per-core pipeline (bf16 I/O to halve HBM traffic; rel err ~4e-3 << 2e-2):
  1. xbar-transpose DMA load: x chunk [2048b x 1024c] -> xT [128c x 8k x 2048b]
     (one contiguous 4MB source per superchunk; HW transposes during DMA).
  2. step1 per (k, j): matmul(lhsT=xT block [c x b], rhs=A_k) -> y batch-major.
     A_k is the 128x128 block-diagonal grouped weight, with output columns
     ordered (v, s_l) so the eviction assembles yB with v-major columns
     (global col = v*128 + s).
  3. tY per v: PE transpose of yB v-block [128b x 128s] -> yT_v [s x b].
  4. step2 per v: matmul(lhsT=yT_v, rhs=kernel_full [s x t]) -> z[b, t]
     batch-major directly; evicted into zB columns t*8+v (stride-8 AP).
  5. contiguous bf16 store.
"""

import os
from contextlib import ExitStack

import numpy as np
import ml_dtypes

import concourse.bass as bass
import concourse.tile as tile
from concourse import bacc, mybir
from concourse.bass_utils import run_bass_kernel_spmd

B, C, W, S = 131072, 1024, 8, 128
NCORES = 8
BSH = B // NCORES          # 16384 rows per core
SC = 2048                  # superchunk rows (one 4MB transposed load)
NSC = BSH // SC            # 8 superchunks
NJ = SC // 128             # 16 batch subtiles per superchunk
NK = C // 128              # 8 channel tiles

F32 = mybir.dt.float32
BF16 = mybir.dt.bfloat16

TRACE = bool(int(os.environ.get("KERNEL_TRACE", "0")))
LAST_EXEC_NS = None
LAST_TRACE_DIR = None

_cache = {}


def _setup_trace_shim():
    """The agent image lacks antenv.axon_hooks; register the NTFF profile
    hook ourselves so run_bass_kernel_spmd(trace=True) works."""
    import sys
    import types

    import antenv
    from trn_agent_boot.trn_boot import _ntff_profile_via_ctypes

    if "antenv.axon_hooks" in sys.modules:
        return
    mod = types.ModuleType("antenv.axon_hooks")
    mod._hook = _ntff_profile_via_ctypes("/opt/axon/libaxon_pjrt.so")
    mod.get_axon_ntff_profile_hook = lambda: mod._hook
    mod.set_axon_ntff_profile_hook = lambda h: setattr(mod, "_hook", h)
    sys.modules["antenv.axon_hooks"] = mod
    antenv.axon_hooks = mod
    # no bucket in this container; keep artifacts local
    import concourse.bass_utils as bu

    bu.upload_artifacts = lambda tmpdir: tmpdir


def _build():
    nc = bacc.Bacc(
        "TRN2", target_bir_lowering=False, debug=False, num_devices=NCORES
    )
    # sc-major + k-major layout: each superchunk is one contiguous 4MB
    # block whose 128-col rows make the xbar transpose DMA's source tile
    # reads contiguous (fast M2S concat path)
    x_ap = nc.dram_tensor(
        "x", [NSC, NK * SC, 128], BF16, kind="ExternalInput"
    ).ap()
    a_ap = nc.dram_tensor("a", [NK, 128, 128], BF16, kind="ExternalInput").ap()
    kf_ap = nc.dram_tensor("kf", [128, 128], BF16, kind="ExternalInput").ap()
    id_ap = nc.dram_tensor("ident", [128, 128], BF16, kind="ExternalInput").ap()
    z_ap = nc.dram_tensor("z", [BSH, C], BF16, kind="ExternalOutput").ap()

    with tile.TileContext(nc) as tc, ExitStack() as ctx:
        consts = ctx.enter_context(tc.tile_pool(name="consts", bufs=1))
        ident = consts.tile([128, 128], BF16)
        nc.sync.dma_start(ident, id_ap)
        a_sb = consts.tile([128, NK, 128], BF16)
        nc.sync.dma_start(a_sb, a_ap.rearrange("k p c -> p k c"))
        kf_sb = consts.tile([128, 128], BF16)
        nc.sync.dma_start(kf_sb, kf_ap)

        xtpool = ctx.enter_context(tc.tile_pool(name="xt", bufs=3))
        ybpool = ctx.enter_context(tc.tile_pool(name="yb", bufs=3))
        ytpool = ctx.enter_context(tc.tile_pool(name="yt", bufs=3))
        zpool = ctx.enter_context(tc.tile_pool(name="zb", bufs=2))
        ps1 = ctx.enter_context(tc.tile_pool(name="ps1", bufs=2, space="PSUM"))
        pst = ctx.enter_context(tc.tile_pool(name="pst", bufs=2, space="PSUM"))
        ps2 = ctx.enter_context(tc.tile_pool(name="ps2", bufs=1, space="PSUM"))

        def load_xt(sc, split=1):
            # split=N issues N transposes so the first compute can start
            # before the whole superchunk has landed (startup latency)
            xt = xtpool.tile([128, NK, SC], BF16, tag="xt", name="xt")
            rows = NK * SC // split
            for i in range(split):
                nc.sync.dma_start_transpose(
                    xt[:, i * NK // split:(i + 1) * NK // split, :],
                    x_ap[sc, i * rows:(i + 1) * rows, :],
                )
            return xt

        def step1(xt, j):
            """grouped matmul for subtile j -> yb [128b x (v, k, s_l)]
            (v-major columns: global col v*128 + 16k + s_l = v*128 + s)"""
            yb = ybpool.tile([128, 8, 8, 16], BF16, tag="yb")  # (v, k, s_l)
            p1 = ps1.tile([128, 8, 8, 16], F32, tag="p1")      # (k, v, s_l)
            for k in range(NK):
                nc.tensor.matmul(
                    p1[:, k, :, :],
                    xt[:, k, j * 128:(j + 1) * 128],
                    a_sb[:, k, :],
                )
            # f32 PSUM reads run at 1 elem/lane/cycle on both engines;
            # ACT takes ~2/3 of the f32 casts, DVE has the bf16 copies
            eng_copy(
                nc.vector if j % 3 == 0 else nc.scalar,
                out=yb,
                in_=p1.rearrange("p k v s -> p v k s"),
            )
            return yb

        def step2(yb, j, zb):
            """transpose each v-block to [s x b], then mix with kf"""
            ybf = yb.rearrange("p v k s -> p v (k s)")
            pt = pst.tile([128, 8, 128], BF16, tag="pt")
            for v in range(8):
                nc.tensor.transpose(pt[:, v, :], ybf[:, v, :], ident)
            yt = ytpool.tile([128, 8, 128], BF16, tag="yt")
            eng_copy(nc.vector, out=yt, in_=pt)  # bf16 copy: 2x only on DVE
            # zb stays (v, t)-major; host does the final (t*8+v) interleave
            for h in range(2):
                p2 = ps2.tile([128, 4, 128], F32, tag=f"p2_{h}", name="p2")
                for v in range(4 * h, 4 * h + 4):
                    nc.tensor.matmul(p2[:, v % 4, :], yt[:, v, :], kf_sb)
                eng_copy(
                    nc.vector if j % 3 == 1 else nc.scalar,
                    out=zb[:, j, 4 * h:4 * h + 4, :],
                    in_=p2,
                )

        def eng_copy(eng, out, in_):
            if eng is nc.scalar:
                eng.copy(out=out, in_=in_)
            else:
                eng.tensor_copy(out=out, in_=in_)

        for sc in range(NSC):
            xt = load_xt(sc, split=4 if sc == 0 else 1)
            zb = zpool.tile([128, NJ, 8, 128], BF16, tag="zb")  # (j, v, t)
            # software-pipelined emission: step1(j+1) is queued on the PE
            # before step2(j) so the PE isn't stalled waiting on evictions
            yb_next = step1(xt, 0)
            for j in range(NJ):
                yb = yb_next
                if j + 1 < NJ:
                    yb_next = step1(xt, j + 1)
                step2(yb, j, zb)
            # stores live on the scalar HWDGE ring; loads own the sync ring
            nc.scalar.dma_start(
                z_ap[sc * SC:(sc + 1) * SC, :].rearrange(
                    "(j p) c -> p j c", p=128
                ),
                zb.rearrange("p j v t -> p j (v t)"),
            )

    nc.compile()
    return nc


def _make_A(ks):
    """A_k[(s_l*8+w), (v*16+s_l)] = ks[16k+s_l, w, v] (block-diag grouped
    weight with (v, s_l)-ordered output columns)."""
    A = np.zeros((NK, 128, 128), np.float32)
    for k in range(NK):
        for sl in range(16):
            A[k, sl * 8:(sl + 1) * 8, sl::16] = ks[16 * k + sl]
    return A


def kernel(x, kernel_seq, kernel_full):
    global LAST_EXEC_NS
    x = np.asarray(x, dtype=np.float32)
    ks = np.asarray(kernel_seq, dtype=np.float32)
    kf = np.asarray(kernel_full, dtype=np.float32)

    xb = np.ascontiguousarray(x).astype(ml_dtypes.bfloat16)
    ab = _make_A(ks).astype(ml_dtypes.bfloat16)
    kfb = np.ascontiguousarray(kf).astype(ml_dtypes.bfloat16)
    ident = np.eye(128, dtype=ml_dtypes.bfloat16)

    if "nc" not in _cache:
        _cache["nc"] = _build()
    nc = _cache["nc"]

    # per-core [NSC, NK*SC, 128]: superchunk-major, k-major within
    xs = np.ascontiguousarray(
        xb.reshape(NCORES, NSC, SC, NK, 128).transpose(0, 1, 3, 2, 4)
    ).reshape(NCORES, NSC, NK * SC, 128)
    in_maps = [
        {"x": xs[i], "a": ab, "kf": kfb, "ident": ident} for i in range(NCORES)
    ]
    kw = {}
    if TRACE:
        _setup_trace_shim()
        global LAST_TRACE_DIR
        import tempfile

        LAST_TRACE_DIR = tempfile.mkdtemp(prefix="ktrace_")
        kw = {"tmpdir": LAST_TRACE_DIR}
    res = run_bass_kernel_spmd(nc, in_maps, list(range(NCORES)), trace=TRACE, **kw)
    if res.exec_time_ns is not None:
        LAST_EXEC_NS = res.exec_time_ns
    z = np.concatenate([r["z"] for r in res.results], axis=0)
    # device stores (v, t)-major columns; natural layout is c = t*8 + v
    z = z.reshape(B, 8, 128).transpose(0, 2, 1).reshape(B, C)
    return np.ascontiguousarray(z.astype(np.float32))


# revision 32
# speedup vs baseline: 1.6130x; 1.0601x over previous
"""Trainium2 Bass kernel for nn_GroupDenseFull.

Math: z[b, t*8+v] = sum_{s,w} x[b, s*8+w] * kernel_seq[s,w,v] * kernel_full[s,t]

Instead of materializing the dense 1024x1024 combined weight (275 GFLOP), use
the factored form (36.5 GFLOP):
  step1 (grouped): y[b,s,v] = sum_w x[b,s,w] * ks[s,w,v]
  step2 (mixing):  z[b,t,v] = sum_s y[b,s,v] * kf[s,t]

Sharding: data-parallel over batch across 8 cores (16384 rows each).

Per-core pipeline (bf16 I/O to halve HBM traffic; rel err ~4e-3 << 2e-2):
  1. xbar-transpose DMA load: x chunk [2048b x 1024c] -> xT [128c x 8k x 2048b]
     (one contiguous 4MB source per superchunk; HW transposes during DMA).
  2. step1 per (k, j): matmul(lhsT=xT block [c x b], rhs=A_k) -> y batch-major.
     A_k is the 128x128 block-diagonal grouped weight, with output columns
     ordered (v, s_l) so the eviction assembles yB with v-major columns
     (global col = v*128 + s).
  3. tY per v: PE transpose of yB v-block [128b x 128s] -> yT_v [s x b].
  4. step2 per v: matmul(lhsT=yT_v, rhs=kernel_full [s x t]) -> z[b, t]
     batch-major directly; evicted into zB columns t*8+v (stride-8 AP).
  5. contiguous bf16 store.
"""

import os
from contextlib import ExitStack

import numpy as np
import ml_dtypes

import concourse.bass as bass
import concourse.tile as tile
from concourse import bacc, mybir
from concourse.bass_utils import run_bass_kernel_spmd

B, C, W, S = 131072, 1024, 8, 128
NCORES = 8
BSH = B // NCORES          # 16384 rows per core
SC = 2048                  # superchunk rows (one 4MB transposed load)
NSC = BSH // SC            # 8 superchunks
NJ = SC // 128             # 16 batch subtiles per superchunk
NK = C // 128              # 8 channel tiles

F32 = mybir.dt.float32
BF16 = mybir.dt.bfloat16

TRACE = bool(int(os.environ.get("KERNEL_TRACE", "0")))
LAST_EXEC_NS = None
LAST_TRACE_DIR = None

_cache = {}


def _setup_trace_shim():
    """The agent image lacks antenv.axon_hooks; register the NTFF profile
    hook ourselves so run_bass_kernel_spmd(trace=True) works."""
    import sys
    import types

    import antenv
    from trn_agent_boot.trn_boot import _ntff_profile_via_ctypes

    if "antenv.axon_hooks" in sys.modules:
        return
    mod = types.ModuleType("antenv.axon_hooks")
    mod._hook = _ntff_profile_via_ctypes("/opt/axon/libaxon_pjrt.so")
    mod.get_axon_ntff_profile_hook = lambda: mod._hook
    mod.set_axon_ntff_profile_hook = lambda h: setattr(mod, "_hook", h)
    sys.modules["antenv.axon_hooks"] = mod
    antenv.axon_hooks = mod
    # no bucket in this container; keep artifacts local
    import concourse.bass_utils as bu

    bu.upload_artifacts = lambda tmpdir: tmpdir


def _build():
    nc = bacc.Bacc(
        "TRN2", target_bir_lowering=False, debug=False, num_devices=NCORES
    )
    # sc-major + k-major layout: each superchunk is one contiguous 4MB
    # block whose 128-col rows make the xbar transpose DMA's source tile
    # reads contiguous (fast M2S concat path)
    x_ap = nc.dram_tensor(
        "x", [NSC, NK * SC, 128], BF16, kind="ExternalInput"
    ).ap()
    a_ap = nc.dram_tensor("a", [NK, 128, 128], BF16, kind="ExternalInput").ap()
    kf_ap = nc.dram_tensor("kf", [128, 128], BF16, kind="ExternalInput").ap()
    id_ap = nc.dram_tensor("ident", [128, 128], BF16, kind="ExternalInput").ap()
    z_ap = nc.dram_tensor("z", [BSH, C], BF16, kind="ExternalOutput").ap()

    with tile.TileContext(nc) as tc, ExitStack() as ctx:
        consts = ctx.enter_context(tc.tile_pool(name="consts", bufs=1))
        ident = consts.tile([128, 128], BF16)
        nc.sync.dma_start(ident, id_ap)
        a_sb = consts.tile([128, NK, 128], BF16)
        nc.sync.dma_start(a_sb, a_ap.rearrange("k p c -> p k c"))
        kf_sb = consts.tile([128, 128], BF16)
        nc.sync.dma_start(kf_sb, kf_ap)

        xtpool = ctx.enter_context(tc.tile_pool(name="xt", bufs=3))
        ybpool = ctx.enter_context(tc.tile_pool(name="yb", bufs=3))
        ytpool = ctx.enter_context(tc.tile_pool(name="yt", bufs=3))
        zpool = ctx.enter_context(tc.tile_pool(name="zb", bufs=2))
        ps1 = ctx.enter_context(tc.tile_pool(name="ps1", bufs=2, space="PSUM"))
        pst = ctx.enter_context(tc.tile_pool(name="pst", bufs=2, space="PSUM"))
        ps2 = ctx.enter_context(tc.tile_pool(name="ps2", bufs=1, space="PSUM"))

        def load_xt(sc, split=1):
            # split=N issues N transposes so the first compute can start
            # before the whole superchunk has landed (startup latency)
            xt = xtpool.tile([128, NK, SC], BF16, tag="xt", name="xt")
            rows = NK * SC // split
            for i in range(split):
                nc.sync.dma_start_transpose(
                    xt[:, i * NK // split:(i + 1) * NK // split, :],
                    x_ap[sc, i * rows:(i + 1) * rows, :],
                )
            return xt

        def step1(xt, j):
            """grouped matmul for subtile j -> yb [128b x (v, k, s_l)]
            (v-major columns: global col v*128 + 16k + s_l = v*128 + s)"""
            yb = ybpool.tile([128, 8, 8, 16], BF16, tag="yb")  # (v, k, s_l)
            p1 = ps1.tile([128, 8, 8, 16], F32, tag="p1")      # (k, v, s_l)
            for k in range(NK):
                nc.tensor.matmul(
                    p1[:, k, :, :],
                    xt[:, k, j * 128:(j + 1) * 128],
                    a_sb[:, k, :],
                )
            # f32 PSUM reads run at 1 elem/lane/cycle on both engines;
            # ACT takes ~2/3 of the f32 casts, DVE has the bf16 copies
            eng_copy(
                nc.vector if j % 3 == 0 else nc.scalar,
                out=yb,
                in_=p1.rearrange("p k v s -> p v k s"),
            )
            return yb

        def step2(yb, j, zb):
            """transpose each v-block to [s x b], then mix with kf"""
            ybf = yb.rearrange("p v k s -> p v (k s)")
            pt = pst.tile([128, 8, 128], BF16, tag="pt")
            for v in range(8):
                nc.tensor.transpose(pt[:, v, :], ybf[:, v, :], ident)
            yt = ytpool.tile([128, 8, 128], BF16, tag="yt")
            eng_copy(nc.vector, out=yt, in_=pt)  # bf16 copy: 2x only on DVE
            # zb stays (v, t)-major; host does the final (t*8+v) interleave
            for h in range(2):
                p2 = ps2.tile([128, 4, 128], F32, tag=f"p2_{h}", name="p2")
                for v in range(4 * h, 4 * h + 4):
                    nc.tensor.matmul(p2[:, v % 4, :], yt[:, v, :], kf_sb)
                eng_copy(
                    nc.vector if j % 3 == 1 else nc.scalar,
                    out=zb[:, j, 4 * h:4 * h + 4, :],
                    in_=p2,
                )

        def eng_copy(eng, out, in_):
            if eng is nc.scalar:
                eng.copy(out=out, in_=in_)
            else:
                eng.tensor_copy(out=out, in_=in_)

        for sc in range(NSC):
            xt = load_xt(sc, split=4 if sc == 0 else 1)
            zb = zpool.tile([128, NJ, 8, 128], BF16, tag="zb")  # (j, v, t)
            # software-pipelined emission: step1(j+1) is queued on the PE
            # before step2(j) so the PE isn't stalled waiting on evictions
            yb_next = step1(xt, 0)
            for j in range(NJ):
                yb = yb_next
                if j + 1 < NJ:
                    yb_next = step1(xt, j + 1)
                step2(yb, j, zb)
            # stores go through SWDGE (gpsimd): keeps DMA waits off the
            # eviction engines and out of the transpose ring's FIFO
            nc.gpsimd.dma_start(
                z_ap[sc * SC:(sc + 1) * SC, :].rearrange(
                    "(j p) c -> p j c", p=128
                ),
                zb.rearrange("p j v t -> p j (v t)"),
            )

    nc.compile()
    return nc


def _make_A(ks):
    """A_k[(s_l*8+w), (v*16+s_l)] = ks[16k+s_l, w, v] (block-diag grouped
    weight with (v, s_l)-ordered output columns)."""
    A = np.zeros((NK, 128, 128), np.float32)
    for k in range(NK):
        for sl in range(16):
            A[k, sl * 8:(sl + 1) * 8, sl::16] = ks[16 * k + sl]
    return A


def kernel(x, kernel_seq, kernel_full):
    global LAST_EXEC_NS
    x = np.asarray(x, dtype=np.float32)
    ks = np.asarray(kernel_seq, dtype=np.float32)
    kf = np.asarray(kernel_full, dtype=np.float32)

    xb = np.ascontiguousarray(x).astype(ml_dtypes.bfloat16)
    ab = _make_A(ks).astype(ml_dtypes.bfloat16)
    kfb = np.ascontiguousarray(kf).astype(ml_dtypes.bfloat16)
    ident = np.eye(128, dtype=ml_dtypes.bfloat16)

    if "nc" not in _cache:
        _cache["nc"] = _build()
    nc = _cache["nc"]

    # per-core [NSC, NK*SC, 128]: superchunk-major, k-major within
    xs = np.ascontiguousarray(
        xb.reshape(NCORES, NSC, SC, NK, 128).transpose(0, 1, 3, 2, 4)
    ).reshape(NCORES, NSC, NK * SC, 128)
    in_maps = [
        {"x": xs[i], "a": ab, "kf": kfb, "ident": ident} for i in range(NCORES)
    ]
    kw = {}
    if TRACE:
        _setup_trace_shim()
        global LAST_TRACE_DIR
        import tempfile

        LAST_TRACE_DIR = tempfile.mkdtemp(prefix="ktrace_")
        kw = {"tmpdir": LAST_TRACE_DIR}
    res = run_bass_kernel_spmd(nc, in_maps, list(range(NCORES)), trace=TRACE, **kw)
    if res.exec_time_ns is not None:
        LAST_EXEC_NS = res.exec_time_ns
    z = np.concatenate([r["z"] for r in res.results], axis=0)
    # device stores (v, t)-major columns; natural layout is c = t*8 + v
    z = z.reshape(B, 8, 128).transpose(0, 2, 1).reshape(B, C)
    return np.ascontiguousarray(z.astype(np.float32))


# revision 34
# speedup vs baseline: 1.7138x; 1.0625x over previous
"""Trainium2 Bass kernel for nn_GroupDenseFull.

Math: z[b, t*8+v] = sum_{s,w} x[b, s*8+w] * kernel_seq[s,w,v] * kernel_full[s,t]

Instead of materializing the dense 1024x1024 combined weight (275 GFLOP), use
the factored form (36.5 GFLOP):
  step1 (grouped): y[b,s,v] = sum_w x[b,s,w] * ks[s,w,v]
  step2 (mixing):  z[b,t,v] = sum_s y[b,s,v] * kf[s,t]

Sharding: data-parallel over batch across 8 cores (16384 rows each).

Per-core pipeline (bf16 I/O to halve HBM traffic; rel err ~4e-3 << 2e-2):
  1. xbar-transpose DMA load: x chunk [2048b x 1024c] -> xT [128c x 8k x 2048b]
     (one contiguous 4MB source per superchunk; HW transposes during DMA).
  2. step1 per (k, j): matmul(lhsT=xT block [c x b], rhs=A_k) -> y batch-major.
     A_k is the 128x128 block-diagonal grouped weight, with output columns
     ordered (v, s_l) so the eviction assembles yB with v-major columns
     (global col = v*128 + s).
  3. tY per v: PE transpose of yB v-block [128b x 128s] -> yT_v [s x b].
  4. step2 per v: matmul(lhsT=yT_v, rhs=kernel_full [s x t]) -> z[b, t]
     batch-major directly; evicted into zB columns t*8+v (stride-8 AP).
  5. contiguous bf16 store.
"""

import os
from contextlib import ExitStack

import numpy as np
import ml_dtypes

import concourse.bass as bass
import concourse.tile as tile
from concourse import bacc, mybir
from concourse.bass_utils import run_bass_kernel_spmd

B, C, W, S = 131072, 1024, 8, 128
NCORES = 8
BSH = B // NCORES          # 16384 rows per core
SC = 2048                  # superchunk rows (one 4MB transposed load)
NSC = BSH // SC            # 8 superchunks
NJ = SC // 128             # 16 batch subtiles per superchunk
NK = C // 128              # 8 channel tiles

F32 = mybir.dt.float32
BF16 = mybir.dt.bfloat16

TRACE = bool(int(os.environ.get("KERNEL_TRACE", "0")))
LAST_EXEC_NS = None
LAST_TRACE_DIR = None

_cache = {}


def _setup_trace_shim():
    """The agent image lacks antenv.axon_hooks; register the NTFF profile
    hook ourselves so run_bass_kernel_spmd(trace=True) works."""
    import sys
    import types

    import antenv
    from trn_agent_boot.trn_boot import _ntff_profile_via_ctypes

    if "antenv.axon_hooks" in sys.modules:
        return
    mod = types.ModuleType("antenv.axon_hooks")
    mod._hook = _ntff_profile_via_ctypes("/opt/axon/libaxon_pjrt.so")
    mod.get_axon_ntff_profile_hook = lambda: mod._hook
    mod.set_axon_ntff_profile_hook = lambda h: setattr(mod, "_hook", h)
    sys.modules["antenv.axon_hooks"] = mod
    antenv.axon_hooks = mod
    # no bucket in this container; keep artifacts local
    import concourse.bass_utils as bu

    bu.upload_artifacts = lambda tmpdir: tmpdir


def _build():
    nc = bacc.Bacc(
        "TRN2", target_bir_lowering=False, debug=False, num_devices=NCORES
    )
    # sc-major + k-major layout: each superchunk is one contiguous 4MB
    # block whose 128-col rows make the xbar transpose DMA's source tile
    # reads contiguous (fast M2S concat path)
    x_ap = nc.dram_tensor(
        "x", [NSC, NK * SC, 128], BF16, kind="ExternalInput"
    ).ap()
    a_ap = nc.dram_tensor("a", [NK, 128, 128], BF16, kind="ExternalInput").ap()
    kf_ap = nc.dram_tensor("kf", [128, 128], BF16, kind="ExternalInput").ap()
    id_ap = nc.dram_tensor("ident", [128, 128], BF16, kind="ExternalInput").ap()
    z_ap = nc.dram_tensor("z", [BSH, C], BF16, kind="ExternalOutput").ap()

    with tile.TileContext(nc) as tc, ExitStack() as ctx:
        consts = ctx.enter_context(tc.tile_pool(name="consts", bufs=1))
        ident = consts.tile([128, 128], BF16)
        nc.sync.dma_start(ident, id_ap)
        a_sb = consts.tile([128, NK, 128], BF16)
        nc.sync.dma_start(a_sb, a_ap.rearrange("k p c -> p k c"))
        kf_sb = consts.tile([128, 128], BF16)
        nc.sync.dma_start(kf_sb, kf_ap)

        xtpool = ctx.enter_context(tc.tile_pool(name="xt", bufs=3))
        ybpool = ctx.enter_context(tc.tile_pool(name="yb", bufs=3))
        ytpool = ctx.enter_context(tc.tile_pool(name="yt", bufs=3))
        zpool = ctx.enter_context(tc.tile_pool(name="zb", bufs=4))
        ps1 = ctx.enter_context(tc.tile_pool(name="ps1", bufs=2, space="PSUM"))
        pst = ctx.enter_context(tc.tile_pool(name="pst", bufs=2, space="PSUM"))
        ps2 = ctx.enter_context(tc.tile_pool(name="ps2", bufs=1, space="PSUM"))

        def load_xt(sc, split=1):
            # split=N issues N transposes so the first compute can start
            # before the whole superchunk has landed (startup latency)
            xt = xtpool.tile([128, NK, SC], BF16, tag="xt", name="xt")
            rows = NK * SC // split
            for i in range(split):
                nc.sync.dma_start_transpose(
                    xt[:, i * NK // split:(i + 1) * NK // split, :],
                    x_ap[sc, i * rows:(i + 1) * rows, :],
                )
            return xt

        def step1(xt, j):
            """grouped matmul for subtile j -> yb [128b x (v, k, s_l)]
            (v-major columns: global col v*128 + 16k + s_l = v*128 + s)"""
            yb = ybpool.tile([128, 8, 8, 16], BF16, tag="yb")  # (v, k, s_l)
            p1 = ps1.tile([128, 8, 8, 16], F32, tag="p1")      # (k, v, s_l)
            for k in range(NK):
                nc.tensor.matmul(
                    p1[:, k, :, :],
                    xt[:, k, j * 128:(j + 1) * 128],
                    a_sb[:, k, :],
                )
            # f32 PSUM reads run at 1 elem/lane/cycle on both engines;
            # ACT takes ~2/3 of the f32 casts, DVE has the bf16 copies
            eng_copy(
                nc.vector if j % 3 == 0 else nc.scalar,
                out=yb,
                in_=p1.rearrange("p k v s -> p v k s"),
            )
            return yb

        def step2(yb, j, zb):
            """transpose each v-block to [s x b], then mix with kf"""
            ybf = yb.rearrange("p v k s -> p v (k s)")
            pt = pst.tile([128, 8, 128], BF16, tag="pt")
            for v in range(8):
                nc.tensor.transpose(pt[:, v, :], ybf[:, v, :], ident)
            yt = ytpool.tile([128, 8, 128], BF16, tag="yt")
            eng_copy(nc.vector, out=yt, in_=pt)  # bf16 copy: 2x only on DVE
            # zb stays (v, t)-major; host does the final (t*8+v) interleave
            for h in range(2):
                p2 = ps2.tile([128, 4, 128], F32, tag=f"p2_{h}", name="p2")
                for v in range(4 * h, 4 * h + 4):
                    nc.tensor.matmul(p2[:, v % 4, :], yt[:, v, :], kf_sb)
                eng_copy(
                    nc.vector if j % 3 == 1 else nc.scalar,
                    out=zb[:, j, 4 * h:4 * h + 4, :],
                    in_=p2,
                )

        def eng_copy(eng, out, in_):
            if eng is nc.scalar:
                eng.copy(out=out, in_=in_)
            else:
                eng.tensor_copy(out=out, in_=in_)

        HJ = NJ // 2  # store granularity: half a superchunk
        for sc in range(NSC):
            xt = load_xt(sc, split=4 if sc == 0 else 1)
            # software-pipelined emission: step1(j+1) is queued on the PE
            # before step2(j) so the PE isn't stalled waiting on evictions
            yb_next = step1(xt, 0)
            for h in range(2):
                zb = zpool.tile([128, HJ, 8, 128], BF16, tag="zb")  # (j,v,t)
                for jj in range(HJ):
                    j = h * HJ + jj
                    yb = yb_next
                    if j + 1 < NJ:
                        yb_next = step1(xt, j + 1)
                    step2(yb, jj, zb)
                # stores go through SWDGE (gpsimd): keeps DMA waits off the
                # eviction engines and out of the transpose ring's FIFO
                r0 = sc * SC + h * HJ * 128
                nc.gpsimd.dma_start(
                    z_ap[r0:r0 + HJ * 128, :].rearrange(
                        "(j p) c -> p j c", p=128
                    ),
                    zb.rearrange("p j v t -> p j (v t)"),
                )

    nc.compile()
    return nc


def _make_A(ks):
    """A_k[(s_l*8+w), (v*16+s_l)] = ks[16k+s_l, w, v] (block-diag grouped
    weight with (v, s_l)-ordered output columns)."""
    A = np.zeros((NK, 128, 128), np.float32)
    for k in range(NK):
        for sl in range(16):
            A[k, sl * 8:(sl + 1) * 8, sl::16] = ks[16 * k + sl]
    return A


def kernel(x, kernel_seq, kernel_full):
    global LAST_EXEC_NS
    x = np.asarray(x, dtype=np.float32)
    ks = np.asarray(kernel_seq, dtype=np.float32)
    kf = np.asarray(kernel_full, dtype=np.float32)

    xb = np.ascontiguousarray(x).astype(ml_dtypes.bfloat16)
    ab = _make_A(ks).astype(ml_dtypes.bfloat16)
    kfb = np.ascontiguousarray(kf).astype(ml_dtypes.bfloat16)
    ident = np.eye(128, dtype=ml_dtypes.bfloat16)

    if "nc" not in _cache:
        _cache["nc"] = _build()
    nc = _cache["nc"]

    # per-core [NSC, NK*SC, 128]: superchunk-major, k-major within
    xs = np.ascontiguousarray(
        xb.reshape(NCORES, NSC, SC, NK, 128).transpose(0, 1, 3, 2, 4)
    ).reshape(NCORES, NSC, NK * SC, 128)
    in_maps = [
        {"x": xs[i], "a": ab, "kf": kfb, "ident": ident} for i in range(NCORES)
    ]
    kw = {}
    if TRACE:
        _setup_trace_shim()
        global LAST_TRACE_DIR
        import tempfile

        LAST_TRACE_DIR = tempfile.mkdtemp(prefix="ktrace_")
        kw = {"tmpdir": LAST_TRACE_DIR}
    res = run_bass_kernel_spmd(nc, in_maps, list(range(NCORES)), trace=TRACE, **kw)
    if res.exec_time_ns is not None:
        LAST_EXEC_NS = res.exec_time_ns
    z = np.concatenate([r["z"] for r in res.results], axis=0)
    # device stores (v, t)-major columns; natural layout is c = t*8 + v
    z = z.reshape(B, 8, 128).transpose(0, 2, 1).reshape(B, C)
    return np.ascontiguousarray(z.astype(np.float32))


# revision 37
# speedup vs baseline: 1.8013x; 1.0510x over previous
"""Trainium2 Bass kernel for nn_GroupDenseFull.

Math: z[b, t*8+v] = sum_{s,w} x[b, s*8+w] * kernel_seq[s,w,v] * kernel_full[s,t]

Instead of materializing the dense 1024x1024 combined weight (275 GFLOP), use
the factored form (36.5 GFLOP):
  step1 (grouped): y[b,s,v] = sum_w x[b,s,w] * ks[s,w,v]
  step2 (mixing):  z[b,t,v] = sum_s y[b,s,v] * kf[s,t]

Sharding: data-parallel over batch across 8 cores (16384 rows each).

Per-core pipeline (bf16 I/O to halve HBM traffic; rel err ~4e-3 << 2e-2):
  1. xbar-transpose DMA load: x chunk [2048b x 1024c] -> xT [128c x 8k x 2048b]
     (one contiguous 4MB source per superchunk; HW transposes during DMA).
  2. step1 per (k, j): matmul(lhsT=xT block [c x b], rhs=A_k) -> y batch-major.
     A_k is the 128x128 block-diagonal grouped weight, with output columns
     ordered (v, s_l) so the eviction assembles yB with v-major columns
     (global col = v*128 + s).
  3. tY per v: PE transpose of yB v-block [128b x 128s] -> yT_v [s x b].
  4. step2 per v: matmul(lhsT=yT_v, rhs=kernel_full [s x t]) -> z[b, t]
     batch-major directly; evicted into zB columns t*8+v (stride-8 AP).
  5. contiguous bf16 store.
"""

import os
from contextlib import ExitStack

import numpy as np
import ml_dtypes

import concourse.bass as bass
import concourse.tile as tile
from concourse import bacc, mybir
from concourse.bass_utils import run_bass_kernel_spmd

B, C, W, S = 131072, 1024, 8, 128
NCORES = 8
BSH = B // NCORES          # 16384 rows per core
SC = 2048                  # superchunk rows (one 4MB transposed load)
NSC = BSH // SC            # 8 superchunks
NJ = SC // 128             # 16 batch subtiles per superchunk
NK = C // 128              # 8 channel tiles

F32 = mybir.dt.float32
BF16 = mybir.dt.bfloat16

TRACE = bool(int(os.environ.get("KERNEL_TRACE", "0")))
LAST_EXEC_NS = None
LAST_TRACE_DIR = None

_cache = {}


def _setup_trace_shim():
    """The agent image lacks antenv.axon_hooks; register the NTFF profile
    hook ourselves so run_bass_kernel_spmd(trace=True) works."""
    import sys
    import types

    import antenv
    from trn_agent_boot.trn_boot import _ntff_profile_via_ctypes

    if "antenv.axon_hooks" in sys.modules:
        return
    mod = types.ModuleType("antenv.axon_hooks")
    mod._hook = _ntff_profile_via_ctypes("/opt/axon/libaxon_pjrt.so")
    mod.get_axon_ntff_profile_hook = lambda: mod._hook
    mod.set_axon_ntff_profile_hook = lambda h: setattr(mod, "_hook", h)
    sys.modules["antenv.axon_hooks"] = mod
    antenv.axon_hooks = mod
    # no bucket in this container; keep artifacts local
    import concourse.bass_utils as bu

    bu.upload_artifacts = lambda tmpdir: tmpdir


def _build():
    nc = bacc.Bacc(
        "TRN2", target_bir_lowering=False, debug=False, num_devices=NCORES
    )
    # sc-major + k-major layout: each superchunk is one contiguous 4MB
    # block whose 128-col rows make the xbar transpose DMA's source tile
    # reads contiguous (fast M2S concat path)
    x_ap = nc.dram_tensor(
        "x", [NSC, NK * SC, 128], BF16, kind="ExternalInput"
    ).ap()
    a_ap = nc.dram_tensor("a", [NK, 128, 128], BF16, kind="ExternalInput").ap()
    kf_ap = nc.dram_tensor("kf", [128, 128], BF16, kind="ExternalInput").ap()
    id_ap = nc.dram_tensor("ident", [128, 128], BF16, kind="ExternalInput").ap()
    # raw tile dump: [half-superchunks, p, j, (v, t)]; host un-permutes.
    # Contiguous 8KB-per-partition writes instead of strided 2KB rows.
    z_ap = nc.dram_tensor(
        "z", [NSC * 2, 128, 8, 1024], BF16, kind="ExternalOutput"
    ).ap()

    with tile.TileContext(nc) as tc, ExitStack() as ctx:
        consts = ctx.enter_context(tc.tile_pool(name="consts", bufs=1))
        ident = consts.tile([128, 128], BF16)
        nc.sync.dma_start(ident, id_ap)
        a_sb = consts.tile([128, NK, 128], BF16)
        nc.sync.dma_start(a_sb, a_ap.rearrange("k p c -> p k c"))
        kf_sb = consts.tile([128, 128], BF16)
        nc.sync.dma_start(kf_sb, kf_ap)

        xtpool = ctx.enter_context(tc.tile_pool(name="xt", bufs=3))
        ybpool = ctx.enter_context(tc.tile_pool(name="yb", bufs=3))
        ytpool = ctx.enter_context(tc.tile_pool(name="yt", bufs=3))
        zpool = ctx.enter_context(tc.tile_pool(name="zb", bufs=4))
        ps1 = ctx.enter_context(tc.tile_pool(name="ps1", bufs=2, space="PSUM"))
        pst = ctx.enter_context(tc.tile_pool(name="pst", bufs=2, space="PSUM"))
        ps2 = ctx.enter_context(tc.tile_pool(name="ps2", bufs=1, space="PSUM"))

        def load_xt(sc, split=1):
            # split=N issues N transposes so the first compute can start
            # before the whole superchunk has landed (startup latency)
            xt = xtpool.tile([128, NK, SC], BF16, tag="xt", name="xt")
            rows = NK * SC // split
            for i in range(split):
                nc.sync.dma_start_transpose(
                    xt[:, i * NK // split:(i + 1) * NK // split, :],
                    x_ap[sc, i * rows:(i + 1) * rows, :],
                )
            return xt

        def step1(xt, j):
            """grouped matmul for subtile j -> yb [128b x (v, k, s_l)]
            (v-major columns: global col v*128 + 16k + s_l = v*128 + s)"""
            yb = ybpool.tile([128, 8, 8, 16], BF16, tag="yb")  # (v, k, s_l)
            p1 = ps1.tile([128, 8, 8, 16], F32, tag="p1")      # (k, v, s_l)
            for k in range(NK):
                nc.tensor.matmul(
                    p1[:, k, :, :],
                    xt[:, k, j * 128:(j + 1) * 128],
                    a_sb[:, k, :],
                )
            # f32 PSUM reads run at 1 elem/lane/cycle on both engines;
            # ACT takes ~2/3 of the f32 casts, DVE has the bf16 copies
            eng_copy(
                nc.vector if j % 3 == 0 else nc.scalar,
                out=yb,
                in_=p1.rearrange("p k v s -> p v k s"),
            )
            return yb

        def step2(yb, j, zb):
            """transpose each v-block to [s x b], then mix with kf"""
            ybf = yb.rearrange("p v k s -> p v (k s)")
            pt = pst.tile([128, 8, 128], BF16, tag="pt")
            for v in range(8):
                nc.tensor.transpose(pt[:, v, :], ybf[:, v, :], ident)
            yt = ytpool.tile([128, 8, 128], BF16, tag="yt")
            eng_copy(nc.vector, out=yt, in_=pt)  # bf16 copy: 2x only on DVE
            # zb stays (v, t)-major; host does the final (t*8+v) interleave
            for h in range(2):
                p2 = ps2.tile([128, 4, 128], F32, tag=f"p2_{h}", name="p2")
                for v in range(4 * h, 4 * h + 4):
                    nc.tensor.matmul(p2[:, v % 4, :], yt[:, v, :], kf_sb)
                eng_copy(
                    nc.vector if j % 3 == 1 else nc.scalar,
                    out=zb[:, j, 4 * h:4 * h + 4, :],
                    in_=p2,
                )

        def eng_copy(eng, out, in_):
            if eng is nc.scalar:
                eng.copy(out=out, in_=in_)
            else:
                eng.tensor_copy(out=out, in_=in_)

        HJ = NJ // 2  # store granularity: half a superchunk
        for sc in range(NSC):
            xt = load_xt(sc, split=4 if sc == 0 else 1)
            # software-pipelined emission: step1(j+1) is queued on the PE
            # before step2(j) so the PE isn't stalled waiting on evictions
            yb_next = step1(xt, 0)
            for h in range(2):
                zb = zpool.tile([128, HJ, 8, 128], BF16, tag="zb")  # (j,v,t)
                for jj in range(HJ):
                    j = h * HJ + jj
                    yb = yb_next
                    if j + 1 < NJ:
                        yb_next = step1(xt, j + 1)
                    step2(yb, jj, zb)
                # stores go through SWDGE (gpsimd): keeps DMA waits off the
                # eviction engines and out of the transpose ring's FIFO
                nc.gpsimd.dma_start(
                    z_ap[sc * 2 + h],
                    zb.rearrange("p j v t -> p j (v t)"),
                )

    nc.compile()
    return nc


def _make_A(ks):
    """A_k[(s_l*8+w), (v*16+s_l)] = ks[16k+s_l, w, v] (block-diag grouped
    weight with (v, s_l)-ordered output columns)."""
    A = np.zeros((NK, 128, 128), np.float32)
    for k in range(NK):
        for sl in range(16):
            A[k, sl * 8:(sl + 1) * 8, sl::16] = ks[16 * k + sl]
    return A


def kernel(x, kernel_seq, kernel_full):
    global LAST_EXEC_NS
    x = np.asarray(x, dtype=np.float32)
    ks = np.asarray(kernel_seq, dtype=np.float32)
    kf = np.asarray(kernel_full, dtype=np.float32)

    xb = np.ascontiguousarray(x).astype(ml_dtypes.bfloat16)
    ab = _make_A(ks).astype(ml_dtypes.bfloat16)
    kfb = np.ascontiguousarray(kf).astype(ml_dtypes.bfloat16)
    ident = np.eye(128, dtype=ml_dtypes.bfloat16)

    if "nc" not in _cache:
        _cache["nc"] = _build()
    nc = _cache["nc"]

    # per-core [NSC, NK*SC, 128]: superchunk-major, k-major within
    xs = np.ascontiguousarray(
        xb.reshape(NCORES, NSC, SC, NK, 128).transpose(0, 1, 3, 2, 4)
    ).reshape(NCORES, NSC, NK * SC, 128)
    in_maps = [
        {"x": xs[i], "a": ab, "kf": kfb, "ident": ident} for i in range(NCORES)
    ]
    kw = {}
    if TRACE:
        _setup_trace_shim()
        global LAST_TRACE_DIR
        import tempfile

        LAST_TRACE_DIR = tempfile.mkdtemp(prefix="ktrace_")
        kw = {"tmpdir": LAST_TRACE_DIR}
    res = run_bass_kernel_spmd(nc, in_maps, list(range(NCORES)), trace=TRACE, **kw)
    if res.exec_time_ns is not None:
        LAST_EXEC_NS = res.exec_time_ns
    # device dumps raw tiles [hsc, p, j, v, t]; row b = hsc*1024 + j*128 + p,
    # natural channel c = t*8 + v
    z = np.stack([r["z"] for r in res.results], axis=0)
    z = z.reshape(NCORES * NSC * 2, 128, 8, 8, 128)
    z = z.transpose(0, 2, 1, 4, 3)  # -> [hsc, j, p, t, v]
    z = np.ascontiguousarray(z, dtype=np.float32).reshape(B, C)
    return z
